# revision 1
# baseline (speedup 1.0000x reference)
"""GQA forward kernel for Trainium2, 8-core tensor-parallel (group-aligned).

Problem: B=2, T=2048, D=2048, 32 Q heads / 8 KV heads, head_dim 64, causal.

Sharding: core c owns KV head c and its 4 Q heads (whole GQA group), both
batches.  Output projection is row-parallel Megatron style: each core
contracts its 256 attention-output channels against its slice of Wo and the
host sums the 8 partial outputs (+ bo).

All matmuls fp16 (fp32 PSUM accumulate).  Host packs every input to the
exact SBUF layout so each DMA moves at full rate.  The attention inner loop
is software-pipelined: scores stream ahead while the AV accumulation lags
K2_AVLAG(=4) tiles behind, so the softmax-normalize chain of the previous
strip (deferred in stages into the current strip) never stalls the PE on
PSUM-buffer reuse.  A fill queue of out-projection tiles and batch-1
projection chunks keeps the PE busy wherever the ACT engine (exp) is the
local bottleneck.
"""

import os

import numpy as np

import concourse.mybir as mybir
import concourse.tile as tile
from concourse import bacc
from concourse import bass_utils

P = 128
B = 2
T = 2048
C = 2048
HD = 64
QH = 32
KVH = 8
G = QH // KVH  # 4
NCORES = 8
TCH = 512  # token chunk for projection phase
QCH = 512  # q chunk for attention phase
KT = C // P  # 16 contraction tiles
NT = T // P  # 16 token tiles
f32 = mybir.dt.float32
f32r = mybir.dt.float32r
fp16 = mybir.dt.float16
ADD = mybir.AluOpType.add
MUL = mybir.AluOpType.mult

_CACHE = {}


def _build():
    nc = bacc.Bacc("TRN2", target_bir_lowering=False, debug=False, num_devices=NCORES)

    xp = nc.dram_tensor("xp", [P, B, KT, T], fp16, kind="ExternalInput")
    wqk = nc.dram_tensor("wqk", [P, KT, 256], fp16, kind="ExternalInput")
    wv = nc.dram_tensor("wv", [P, KT, 128], fp16, kind="ExternalInput")
    wo = nc.dram_tensor("wo", [P, 2, C], fp16, kind="ExternalInput")
    bqk = nc.dram_tensor("bqk", [P, 2], f32, kind="ExternalInput")
    bv = nc.dram_tensor("bv", [1, 128], f32, kind="ExternalInput")
    maskd = nc.dram_tensor("mask", [P, 2, P], fp16, kind="ExternalInput")
    y = nc.dram_tensor("y", [B, T, C], fp16, kind="ExternalOutput")

    with tile.TileContext(nc) as tc:
        with (
            tc.tile_pool(name="const", bufs=1) as cpool,
            tc.tile_pool(name="x", bufs=int(os.environ.get("K2_XB", "5"))) as xpool,
            tc.tile_pool(name="proj", bufs=2) as projpool,
            tc.tile_pool(name="attn", bufs=2) as apool,
            tc.tile_pool(name="work", bufs=int(os.environ.get("K2_AVLAG", "4")) + int(os.environ.get("K2_WEX", "2"))) as wpool,
            tc.tile_pool(name="work2", bufs=int(os.environ.get("K2_W2B", "2"))) as wpool2,
            tc.tile_pool(name="psA", bufs=2, space="PSUM") as psumA,
            tc.tile_pool(name="psB", bufs=2, space="PSUM") as psumB,
            tc.tile_pool(name="psC", bufs=2, space="PSUM") as psumC,
        ):
            # ---- startup-critical loads: k-ascending pieces of wqk + x(b0,c0)
            wqk_sb = cpool.tile([P, KT, 256], fp16)
            xch0 = xpool.tile([P, KT, TCH], fp16, tag="xch", name="xch")
            # xch pieces go through the Pool SWDGE path so their dispatch
            # overlaps the SP/HWDGE dispatch of the wqk pieces
            wv_sb = cpool.tile([P, KT, 128], fp16)
            for k0, k1 in ((0, 2), (2, 4), (4, 8), (8, 12), (12, 16)):
                nc.sync.dma_start(wqk_sb[:, k0:k1, :], wqk.ap()[:, k0:k1, :])
                nc.gpsimd.dma_start(xch0[:, k0:k1, :], xp.ap()[:, 0, k0:k1, 0:TCH])
                nc.sync.dma_start(wv_sb[:, k0:k1, :], wv.ap()[:, k0:k1, :])
            bqk_sb = cpool.tile([P, 2], f32)
            nc.sync.dma_start(bqk_sb[:], bqk.ap())
            bv_sb = cpool.tile([P, 128], f32)
            nc.sync.dma_start(bv_sb[:], bv.ap().to_broadcast((P, 128)))
            mask2_sb = cpool.tile([P, 2, P], fp16)
            nc.sync.dma_start(mask2_sb[:], maskd.ap())
            mask_sb = mask2_sb[:, 0, :]
            ones_f = cpool.tile([P, 1], f32)
            nc.gpsimd.memset(ones_f[:], 1.0)
            ones_sb = cpool.tile([P, HD], f32r)
            nc.vector.tensor_copy(ones_sb[:], ones_f[:].to_broadcast((P, HD)))
            ones_r = ones_sb
            wo_sb = cpool.tile([P, 2, C], fp16)

            # ---------- P1 emitters ----------
            def emit_qkk_sub(qkk_sb, xch, tch, sub):
                tsl = slice(tch * TCH, (tch + 1) * TCH)
                pp = psumC.tile([P, TCH], f32, tag="pp", name="pp")
                for k in range(KT):
                    nc.tensor.matmul(
                        pp[:],
                        wqk_sb[:, k, sub * P : (sub + 1) * P],
                        xch[:, k, :],
                        start=(k == 0),
                        stop=(k == KT - 1),
                    )
                nc.vector.tensor_tensor(
                    qkk_sb[:, sub, tsl],
                    pp[:],
                    bqk_sb[:, sub : sub + 1].to_broadcast((P, TCH)),
                    ADD,
                )

            def emit_v_quad(v3_sb, qkk_sb, xch, tch):
                # joint [K|V] projection in natural [t, d] orientation (one
                # bank per group: PSUM start zeroing is bank-granular), then
                # K^T is rebuilt duplicated to both partition halves via two
                # PE transposes
                def do_kt(tidx, kvn):
                    pt = psumC.tile([P, TCH], f32, tag="pp", name="pt")
                    pt16 = pt[:, 0:64].bitcast(fp16)
                    nc.tensor.matmul(
                        pt16[0:64, :], kvn[:, 0:64], mask2_sb[:, 1, :],
                        start=True, stop=True, is_transpose=True,
                    )
                    nc.tensor.matmul(
                        pt16[64:128, :], kvn[:, 0:64], mask2_sb[:, 1, :],
                        start=True, stop=True, is_transpose=True,
                    )
                    nc.vector.tensor_copy(
                        qkk_sb[:, 2, tidx * P : (tidx + 1) * P], pt16[:]
                    )

                kts = []
                for ts in range(TCH // P):
                    tidx = tch * (TCH // P) + ts
                    pv = psumC.tile([P, TCH], f32, tag="pp", name="pv")
                    for k in range(KT):
                        nc.tensor.matmul(
                            pv[:, 0:128],
                            xch[:, k, ts * P : (ts + 1) * P],
                            wv_sb[:, k, :],
                            start=(k == 0),
                            stop=(k == KT - 1),
                        )
                    kvn = wpool2.tile([P, 128], fp16, tag="kvn", name="kvn", bufs=4)
                    nc.vector.tensor_tensor(kvn[:], pv[:, 0:128], bv_sb[:], ADD)
                    nc.vector.tensor_copy(v3_sb[:, tidx, 0:HD], kvn[:, 64:128])
                    kts.append((tidx, kvn))
                for args in kts:
                    do_kt(*args)

            def emit_p1_chunk0(qkk_sb, v3_sb):
                # k-outer over the three QKK subs so the PE can start on the
                # first DMA pieces (third psum group borrows a psumB slot).
                pp0 = psumC.tile([P, TCH], f32, tag="pp", name="pp0")
                pp1 = psumC.tile([P, TCH], f32, tag="pp", name="pp1")
                pps = [pp0, pp1]
                pb0 = psumB.tile([P, QCH], f32, tag="pav", name="pb0")
                pb1 = psumB.tile([P, QCH], f32, tag="pav", name="pb1")
                pa = psumA.tile([P, 2, QCH], f32, tag="ps", name="pa")
                pvs = [pb0[:, 0:128], pb1[:, 0:128], pa[:, 0, 0:128], pa[:, 1, 0:128]]
                for k in range(KT):
                    for sub in range(2):
                        nc.tensor.matmul(
                            pps[sub][:],
                            wqk_sb[:, k, sub * P : (sub + 1) * P],
                            xch0[:, k, :],
                            start=(k == 0),
                            stop=(k == KT - 1),
                            skip_group_check=True,
                        )
                    for ts in range(TCH // P):
                        nc.tensor.matmul(
                            pvs[ts],
                            xch0[:, k, ts * P : (ts + 1) * P],
                            wv_sb[:, k, :],
                            start=(k == 0),
                            stop=(k == KT - 1),
                            skip_group_check=True,
                        )
                for sub in range(2):
                    nc.vector.tensor_tensor(
                        qkk_sb[:, sub, 0:TCH],
                        pps[sub][:],
                        bqk_sb[:, sub : sub + 1].to_broadcast((P, TCH)),
                        ADD,
                    )
                kts = []
                for ts in range(TCH // P):
                    kvn = wpool2.tile([P, 128], fp16, tag="kvn", name="kvn", bufs=4)
                    nc.vector.tensor_tensor(kvn[:], pvs[ts], bv_sb[:], ADD)
                    nc.vector.tensor_copy(v3_sb[:, ts, 0:HD], kvn[:, 64:128])
                    kts.append((ts, kvn))
                for tidx, kvn in kts:
                    pt = psumC.tile([P, TCH], f32, tag="pp", name="pt")
                    pt16 = pt[:, 0:64].bitcast(fp16)
                    nc.tensor.matmul(
                        pt16[0:64, :], kvn[:, 0:64], mask2_sb[:, 1, :],
                        start=True, stop=True, is_transpose=True,
                    )
                    nc.tensor.matmul(
                        pt16[64:128, :], kvn[:, 0:64], mask2_sb[:, 1, :],
                        start=True, stop=True, is_transpose=True,
                    )
                    nc.vector.tensor_copy(
                        qkk_sb[:, 2, tidx * P : (tidx + 1) * P], pt16[:]
                    )

            # ---------- P3 (out-proj) fill units ----------
            def make_p3_units(b, qc, attn_sb, alt_psum=False, eager=False):
                units = []
                state = {}
                for ts in range(qc * (QCH // P), (qc + 1) * (QCH // P)):
                    for ec in range(C // QCH):
                        def unit(ts=ts, ec=ec):
                            if ec == 0:
                                state["ysb"] = wpool2.tile(
                                    [P, C], fp16, tag="ysb", name="ysb",
                                    bufs=int(os.environ.get("K2_YSB", "5")),
                                )
                            if alt_psum and ec % 2 == 1:
                                py = psumA.tile(
                                    [P, 2, QCH], f32, tag="ps", name="py2"
                                )[:, 0, :]
                            else:
                                py = psumC.tile([P, QCH], f32, tag="pp", name="py")
                            for ks in range(2):
                                nc.tensor.matmul(
                                    py[:],
                                    attn_sb[:, ks, ts * P : (ts + 1) * P],
                                    wo_sb[:, ks, ec * QCH : (ec + 1) * QCH],
                                    start=(ks == 0),
                                    stop=(ks == 1),
                                    skip_group_check=True,
                                )
                            # alternate the drain between DVE and ACT to
                            # balance the two engines' queues
                            if ec % 2 == 0:
                                nc.vector.tensor_copy(
                                    state["ysb"][:, ec * QCH : (ec + 1) * QCH], py[:]
                                )
                            else:
                                nc.scalar.copy(
                                    state["ysb"][:, ec * QCH : (ec + 1) * QCH], py[:]
                                )
                            if eager and ts >= qc * (QCH // P) + 2:
                                # last two tiles: stream each 512-wide piece as
                                # soon as it lands (alternating queues) so only
                                # a small transfer remains after the last copy
                                eng = nc.gpsimd if ec % 2 == 0 else nc.sync
                                eng.dma_start(
                                    y.ap()[
                                        b,
                                        ts * P : (ts + 1) * P,
                                        ec * QCH : (ec + 1) * QCH,
                                    ],
                                    state["ysb"][:, ec * QCH : (ec + 1) * QCH],
                                )
                            elif ec == C // QCH - 1:
                                nc.sync.dma_start(
                                    y.ap()[b, ts * P : (ts + 1) * P, :], state["ysb"][:]
                                )
                        units.append(unit)
                return units

            # ---------- fill queue ----------
            fill = []
            RESERVE = int(os.environ.get("K2_RESERVE", "0"))
            STARTPOPS = int(os.environ.get("K2_STARTPOPS", "2"))
            POPMOD = int(os.environ.get("K2_POPMOD", "2"))

            def pop_fill(force=False):
                if fill and (force or len(fill) > RESERVE):
                    fill.pop(0)()

            def drain_fill():
                while fill:
                    fill.pop(0)()

            # ---------- P2: one strip = (batch, sub-pair, q-chunk) ----------
            pending = []  # deferred normalize stages from the previous strip

            def make_norm_stages(b, sub, qc, pav0, pav1, attn_sb, shift=False):
                q0 = qc * QCH

                def stage1():  # denominator rows to SBUF
                    den = wpool2.tile([P, 2, QCH], f32r, tag="den", name="den", bufs=3)
                    nc.vector.tensor_copy(den[64:65, 0, :], pav0[64:65, :])
                    if os.environ.get("K2_DENSPLIT", "0") == "1":
                        nc.scalar.copy(den[64:65, 1, :], pav1[64:65, :])
                    else:
                        nc.vector.tensor_copy(den[64:65, 1, :], pav1[64:65, :])
                    state["den"] = den

                def stage2():  # replicate denominators to partitions 0-63
                    den = state["den"]
                    pr = psumC.tile([P, QCH], f32, tag="pp", name="pr")
                    nc.tensor.matmul(
                        pr[0:64, :], ones_r[64:65, 0:64], den[64:65, 0, :],
                        start=True, stop=True,
                    )
                    pr2 = psumC.tile([P, QCH], f32, tag="pp", name="pr2")
                    nc.tensor.matmul(
                        pr2[0:64, :], ones_r[64:65, 0:64], den[64:65, 1, :],
                        start=True, stop=True,
                    )
                    state["pr"] = pr
                    state["pr2"] = pr2

                def stage3():  # reciprocals (PSUM -> SBUF)
                    rec = wpool2.tile([64, 2, QCH], f32, tag="rec", name="rec", bufs=3)
                    nc.vector.reciprocal(rec[:, 0, :], state["pr"][0:64, :])
                    nc.vector.reciprocal(rec[:, 1, :], state["pr2"][0:64, :])
                    state["rec"] = rec

                def stage4():  # normalized attention rows
                    rec = state["rec"]
                    nc.vector.tensor_tensor(
                        attn_sb[0:64, sub, q0 : q0 + QCH], pav0[0:64, :],
                        rec[:, 0, :], MUL,
                    )
                    alo = wpool2.tile([64, QCH], fp16, tag="alo", name="alo", bufs=3)
                    nc.vector.tensor_tensor(alo[:], pav1[0:64, :], rec[:, 1, :], MUL)
                    if shift:
                        # terminal strips: move partitions via a PE identity
                        # matmul + copy instead of the slower SBUF-SBUF DMA
                        psh = psumC.tile([P, QCH], f32, tag="pp", name="psh")
                        nc.tensor.matmul(
                            psh[64:128, :], mask2_sb[0:64, 1, 0:64], alo[:],
                            start=True, stop=True,
                        )
                        nc.vector.tensor_copy(
                            attn_sb[64:128, sub, q0 : q0 + QCH], psh[64:128, :]
                        )
                    else:
                        nc.sync.dma_start(attn_sb[64:128, sub, q0 : q0 + QCH], alo[:])

                state = {}
                return [stage1, stage2, stage3, stage4]

            def emit_strip(b, sub, qc, qkk_sb, v3_sb, attn_sb):
                q0 = qc * QCH
                nfull = q0 // P
                ntiles = nfull + QCH // P
                order = list(range(ntiles))
                pav0 = psumB.tile([P, QCH], f32, tag="pav", name="pav0")
                pav1 = psumB.tile([P, QCH], f32, tag="pav", name="pav1")
                AVLAG = int(os.environ.get("K2_AVLAG", "4"))
                avq = []

                def flush_av(last=False):
                    pi, pnsl, pexpS, pfirst = avq.pop(0)
                    for half, pav in ((0, pav0), (1, pav1)):
                        nc.tensor.matmul(
                            pav[0:65, pnsl],
                            v3_sb[:, pi, 0:65],
                            pexpS[:, half, pnsl],
                            start=pfirst,
                            stop=last,
                            skip_group_check=True,
                        )

                for step, i in enumerate(order):
                    nsl = slice(0, QCH) if i < nfull else slice((i - nfull) * P, QCH)
                    ksl = slice(i * P, (i + 1) * P)
                    ps_s = psumA.tile([P, 2, QCH], f32, tag="ps", name="ps_s")
                    nc.tensor.matmul(
                        ps_s[:, 0, nsl],
                        qkk_sb[0:64, 2, ksl],
                        qkk_sb[0:64, sub, q0 + nsl.start : q0 + QCH],
                        start=True,
                        stop=True,
                    )
                    nc.tensor.matmul(
                        ps_s[:, 1, nsl],
                        qkk_sb[64:128, 2, ksl],
                        qkk_sb[64:128, sub, q0 + nsl.start : q0 + QCH],
                        start=True,
                        stop=True,
                    )
                    expS = wpool.tile([P, 2, QCH], fp16, tag="expS", name="expS")
                    nc.scalar.activation(
                        expS[:, :, nsl],
                        ps_s[:, :, nsl],
                        mybir.ActivationFunctionType.Exp,
                        scale=0.125,
                    )
                    if i >= nfull:
                        j = i - nfull
                        nc.vector.tensor_tensor(
                            expS[:, :, j * P : (j + 1) * P],
                            expS[:, :, j * P : (j + 1) * P],
                            mask_sb[:, None, :].to_broadcast((P, 2, P)),
                            MUL,
                        )
                    if pending:
                        pending.pop(0)()
                        pop_fill()
                    avq.append((i, nsl, expS, step == 0))
                    if len(avq) > AVLAG:
                        flush_av()
                    if step < int(os.environ.get("K2_STARTRANGE", "3")):
                        for _ in range(STARTPOPS):
                            pop_fill(force=True)
                    elif step % POPMOD == 0:
                        pop_fill()
                while len(avq) > 1:
                    flush_av()
                flush_av(last=True)
                pending.extend(make_norm_stages(b, sub, qc, pav0, pav1, attn_sb, shift=(b == 1 and qc == 3 and sub >= int(os.environ.get("K2_SHIFTSUB", "2")))))

            def drain_pending():
                while pending:
                    pending.pop(0)()
                    pop_fill(force=True)

            # ---------- program ----------
            qkk = {}
            v3 = {}
            attn = {}
            xchs = {}

            # P1(b=0)
            qkk[0] = projpool.tile([P, 3, T], fp16, tag="qkk", name="qkk0")
            v3[0] = projpool.tile([P, NT, 65], fp16, tag="v3", name="v30")
            nc.gpsimd.memset(v3[0][:, :, 64:65], 1.0)
            emit_p1_chunk0(qkk[0], v3[0])
            for tch in range(1, T // TCH):
                xch = xpool.tile([P, KT, TCH], fp16, tag="xch", name="xch")
                tsl = slice(tch * TCH, (tch + 1) * TCH)
                if tch == 1:
                    nc.sync.dma_start(xch[:, 0:8, :], xp.ap()[:, 0, 0:8, tsl])
                    nc.sync.dma_start(xch[:, 8:16, :], xp.ap()[:, 0, 8:16, tsl])
                else:
                    nc.sync.dma_start(xch[:], xp.ap()[:, 0, :, tsl])
                if tch == 2:
                    nc.sync.dma_start(wo_sb[:], wo.ap())
                for sub in range(2):
                    emit_qkk_sub(qkk[0], xch, tch, sub)
                emit_v_quad(v3[0], qkk[0], xch, tch)

            # prefetch b=1's x; build its P1 as fill units
            qkk[1] = projpool.tile([P, 3, T], fp16, tag="qkk", name="qkk1")
            v3[1] = projpool.tile([P, NT, 65], fp16, tag="v3", name="v31")
            nc.gpsimd.memset(v3[1][:, :, 64:65], 1.0)
            for tch in range(2):
                xch = xpool.tile([P, KT, TCH], fp16, tag="xch", name="xch")
                nc.sync.dma_start(xch[:], xp.ap()[:, 1, :, tch * TCH : (tch + 1) * TCH])
                xchs[tch] = xch

            def p1b1_units(tch):
                xch = xchs[tch]
                us = [
                    (lambda sub=sub: emit_qkk_sub(qkk[1], xch, tch, sub))
                    for sub in range(2)
                ]
                us.append(lambda: emit_v_quad(v3[1], qkk[1], xch, tch))
                return us

            # P2/P3 (b=0): fill = P1(b=1) chunk 0 + b=0 out-proj tiles
            attn[0] = apool.tile([P, 2, T], fp16, tag="attn", name="attn0")
            heldover0 = []
            c0units = p1b1_units(0)
            for qc in range(T // QCH):
                if qc < len(c0units):
                    fill.append(c0units[qc])
                if qc >= 2:
                    xch = xpool.tile([P, KT, TCH], fp16, tag="xch", name="xch")
                    nc.sync.dma_start(
                        xch[:], xp.ap()[:, 1, :, qc * TCH : (qc + 1) * TCH]
                    )
                    xchs[qc] = xch
                fill.extend(heldover0)
                heldover0 = []
                emit_strip(0, 0, qc, qkk[0], v3[0], attn[0])
                emit_strip(0, 1, qc, qkk[0], v3[0], attn[0])
                if qc > 0:
                    us = make_p3_units(0, qc - 1, attn[0])
                    h0 = 16 - int(os.environ.get("K2_HOLD0", "0"))
                    fill.extend(us[:h0])
                    heldover0 = us[h0:]
            drain_pending()
            fill.extend(heldover0)
            tailb0 = make_p3_units(0, 3, attn[0])

            # P2/P3 (b=1): strip qc only needs projection chunks <= qc, so
            # P1(b=1) chunks c1..c3 serve as fill inside qc0..qc2
            attn[1] = apool.tile([P, 2, T], fp16, tag="attn", name="attn1")
            heldover = []
            for qc in range(T // QCH):
                if qc == 0:
                    fill[:0] = p1b1_units(1)
                elif qc < 3:
                    fill[:0] = p1b1_units(qc + 1)
                else:
                    fill.extend(tailb0)
                fill.extend(heldover)
                heldover = []
                emit_strip(1, 0, qc, qkk[1], v3[1], attn[1])
                emit_strip(1, 1, qc, qkk[1], v3[1], attn[1])
                if qc > 0:
                    us = make_p3_units(1, qc - 1, attn[1])
                    h1 = 16 - int(os.environ.get("K2_HOLD1", "8"))
                    fill.extend(us[:h1])
                    heldover = us[h1:]
            fill.extend(heldover)
            drain_pending()
            drain_fill()
            for u in make_p3_units(1, 3, attn[1], alt_psum=True, eager=True):
                u()

    nc.compile()
    return nc


def _prep_inputs(x, Wq, bq, Wk, bk, Wv, bv, Wo, bo):
    x = np.ascontiguousarray(np.asarray(x, dtype=np.float32))
    # xp[p, b, ko, t] = x[b, t, ko*128+p]
    xp = np.ascontiguousarray(
        x.transpose(0, 2, 1).reshape(B, KT, P, T).transpose(2, 0, 1, 3)
    ).astype(np.float16)
    Wq = np.asarray(Wq, dtype=np.float32)
    Wk = np.asarray(Wk, dtype=np.float32)
    Wv = np.asarray(Wv, dtype=np.float32)
    Wo = np.asarray(Wo, dtype=np.float32)
    bq = np.asarray(bq, dtype=np.float32)
    bk = np.asarray(bk, dtype=np.float32)
    bv = np.asarray(bv, dtype=np.float32)

    # page 0: mask[kj, qi] = 1 iff kj <= qi (upper triangular incl. diag);
    # page 1: identity (for PE partition-shift copies)
    mask = np.ascontiguousarray(
        np.stack(
            [np.triu(np.ones((P, P), np.float16)), np.eye(P, dtype=np.float16)],
            axis=1,
        )
    )
    in_maps = []
    for c in range(NCORES):
        qs = slice(c * G * HD, (c + 1) * G * HD)
        ks = slice(c * HD, (c + 1) * HD)
        wqk_p = Wq[:, qs].reshape(KT, P, 256).transpose(1, 0, 2)
        wkv_c = np.concatenate([Wk[:, ks], Wv[:, ks]], axis=1)
        wv_p = wkv_c.reshape(KT, P, 128).transpose(1, 0, 2)
        wo_p = Wo[qs, :].reshape(2, P, C).transpose(1, 0, 2)
        bq_c = bq[qs]
        bqk_c = np.stack([bq_c[0:128], bq_c[128:256]], axis=1)
        bkv_c = np.concatenate([bk[ks], bv[ks]])
        in_maps.append(
            {
                "xp": xp,
                "wqk": np.ascontiguousarray(wqk_p).astype(np.float16),
                "wv": np.ascontiguousarray(wv_p).astype(np.float16),
                "wo": np.ascontiguousarray(wo_p).astype(np.float16),
                "bqk": np.ascontiguousarray(bqk_c),
                "bv": np.ascontiguousarray(bkv_c[None, :]),
                "mask": mask,
            }
        )
    return in_maps


def kernel(x, Wq, bq, Wk, bk, Wv, bv, Wo, bo, _trace=False):
    if not _trace:
        os.environ["BASS_NEVER_TRACE"] = "1"
    if "nc" not in _CACHE:
        _CACHE["nc"] = _build()
    nc = _CACHE["nc"]
    in_maps = _prep_inputs(x, Wq, bq, Wk, bk, Wv, bv, Wo, bo)
    res = bass_utils.run_bass_kernel_spmd(
        nc, in_maps, core_ids=list(range(NCORES)), trace=_trace
    )
    bo = np.asarray(bo, dtype=np.float32)
    y = np.zeros((B, T, C), dtype=np.float32)
    for c in range(NCORES):
        y += res.results[c]["y"].astype(np.float32)
    y += bo
    if _trace:
        return y, res
    return y



# revision 9
# speedup vs baseline: 1.0977x; 1.0977x over previous
"""GQA forward kernel for Trainium2, 8-core tensor-parallel (group-aligned).

Problem: B=2, T=2048, D=2048, 32 Q heads / 8 KV heads, head_dim 64, causal.

Sharding: core c owns KV head c and its 4 Q heads (whole GQA group), both
batches.  Output projection is row-parallel Megatron style: each core
contracts its 256 attention-output channels against its slice of Wo and the
host sums the 8 partial outputs (+ bo).

v2 vs the fp16 baseline:
- Q/K/V projections run as 3-term hi/lo fp8e4m3 DoubleRow matmuls
  ((x_hi+x_lo)@(W_hi+W_lo) dropping lo@lo), 0.75x the fp16 PE column count.
  Host prescales x by 8 and W by 64 so both quantize in e4m3's normal
  range; the 512x product scale rides through scores (exp scale absorbs
  it) and AV (the v3 ones-column is 512 so the softmax denominator
  cancels it).
- AV is re-oriented to out[query, 65] (lhsT = expS tile, rhs = [V|512]):
  full 128-partition output use halves the PE time, and the softmax
  denominator lands per-partition so normalization is one reciprocal +
  one broadcast multiply instead of the PE-replication chain.
- Normalized attention is transposed back to [chan, token] with PE
  identity-matmul transposes for the fp16 out-projection.
- P3 drains rotate across DVE/ACT/Pool to keep every engine under the PE
  roofline.
"""

import os

import numpy as np
import ml_dtypes

import concourse.mybir as mybir
import concourse.tile as tile
from concourse import bacc
from concourse import bass_utils

P = 128
B = 2
T = 2048
C = 2048
HD = 64
QH = 32
KVH = 8
G = QH // KVH  # 4
NCORES = 8
TCH = 512  # token chunk for projection phase
QCH = 512  # q chunk for attention phase
KT = C // P  # 16 contraction tiles
NT = T // P  # 16 token tiles
NDR = 24  # DoubleRow matmuls per projection output tile (3 terms x 8 pairs)
SCL = 0.125 / (512.0 * 512.0)  # exp scale: 1/sqrt(64) / (8*64)^2
f32 = mybir.dt.float32
fp16 = mybir.dt.float16
e4 = mybir.dt.float8e4
DRM = mybir.MatmulPerfMode.DoubleRow
ADD = mybir.AluOpType.add
MUL = mybir.AluOpType.mult

_CACHE = {}


def _build():
    nc = bacc.Bacc("TRN2", target_bir_lowering=False, debug=False, num_devices=NCORES)

    # xp8[p, b, j, k, t] = e4m3 of hi/lo split of 8*x[b, t, k*128+p]
    xp8 = nc.dram_tensor("xp8", [P, B, 2, KT, T], e4, kind="ExternalInput")
    # wq8[p, widx, e, m]: widx 0..7 = (64*Wq)_hi k-pairs, 8..15 = lo k-pairs
    wq8 = nc.dram_tensor("wq8", [P, 16, 2, 256], e4, kind="ExternalInput")
    wkv8 = nc.dram_tensor("wkv8", [P, 16, 2, 128], e4, kind="ExternalInput")
    wo = nc.dram_tensor("wo", [P, 2, C], fp16, kind="ExternalInput")
    bqk = nc.dram_tensor("bqk", [P, 2], f32, kind="ExternalInput")
    bkv = nc.dram_tensor("bkv", [1, 128], f32, kind="ExternalInput")
    maskd = nc.dram_tensor("mask", [P, 2, P], fp16, kind="ExternalInput")
    y = nc.dram_tensor("y", [B, T, C], fp16, kind="ExternalOutput")

    AVLAG = int(os.environ.get("K2_AVLAG", "3"))
    WEX = int(os.environ.get("K2_WEX", "2"))

    with tile.TileContext(nc) as tc:
        with (
            tc.tile_pool(name="const", bufs=1) as cpool,
            tc.tile_pool(name="x", bufs=int(os.environ.get("K2_XB", "5"))) as xpool,
            tc.tile_pool(name="proj", bufs=2) as projpool,
            tc.tile_pool(name="attn", bufs=2) as apool,
            tc.tile_pool(name="work", bufs=AVLAG + WEX) as wpool,
            tc.tile_pool(name="work2", bufs=int(os.environ.get("K2_W2B", "2"))) as wpool2,
            tc.tile_pool(name="psA", bufs=2, space="PSUM") as psumA,
            tc.tile_pool(name="psB", bufs=1, space="PSUM") as psumB,
            tc.tile_pool(name="psC", bufs=2, space="PSUM") as psumC,
        ):
            # ---- startup-critical loads: k-ascending pieces of weights + x(b0,c0)
            wq_sb = cpool.tile([P, 16, 2, 256], e4)
            wkv_sb = cpool.tile([P, 16, 2, 128], e4)
            xch0 = xpool.tile([P, 2, KT, TCH], e4, tag="xch", name="xch")
            # xch pieces go through the Pool SWDGE path so their dispatch
            # overlaps the SP/HWDGE dispatch of the weight pieces
            for j0, j1 in ((0, 1), (1, 2), (2, 4), (4, 6), (6, 8)):
                nc.sync.dma_start(wq_sb[:, j0:j1, :, :], wq8.ap()[:, j0:j1, :, :])
                nc.sync.dma_start(
                    wq_sb[:, 8 + j0 : 8 + j1, :, :], wq8.ap()[:, 8 + j0 : 8 + j1, :, :]
                )
                for jj in range(2):
                    nc.gpsimd.dma_start(
                        xch0[:, jj, 2 * j0 : 2 * j1, :],
                        xp8.ap()[:, 0, jj, 2 * j0 : 2 * j1, 0:TCH],
                    )
                nc.sync.dma_start(wkv_sb[:, j0:j1, :, :], wkv8.ap()[:, j0:j1, :, :])
                nc.sync.dma_start(
                    wkv_sb[:, 8 + j0 : 8 + j1, :, :], wkv8.ap()[:, 8 + j0 : 8 + j1, :, :]
                )
            bqk_sb = cpool.tile([P, 2], f32)
            nc.sync.dma_start(bqk_sb[:], bqk.ap())
            bkv_sb = cpool.tile([P, 128], f32)
            nc.sync.dma_start(bkv_sb[:], bkv.ap().to_broadcast((P, 128)))
            mask2_sb = cpool.tile([P, 2, P], fp16)
            nc.sync.dma_start(mask2_sb[:], maskd.ap())
            mask_sb = mask2_sb[:, 0, :]
            wo_sb = cpool.tile([P, 2, C], fp16)

            # DR term schedule: (weight row block, x hi/lo page)
            TERMS = ((0, 0), (0, 1), (8, 0))  # (Whi,xhi), (Whi,xlo), (Wlo,xhi)

            # ---------- P1 emitters ----------
            def emit_q_sub(qkk_sb, xch, tch, sub):
                tsl = slice(tch * TCH, (tch + 1) * TCH)
                pp = psumC.tile([P, TCH], f32, tag="pp", name="pp")
                idx = 0
                for wb, xj in TERMS:
                    for j in range(8):
                        nc.tensor.matmul(
                            pp[:],
                            wq_sb[:, wb + j, :, sub * P : (sub + 1) * P],
                            xch[:, xj, 2 * j : 2 * j + 2, :],
                            start=(idx == 0),
                            stop=(idx == NDR - 1),
                            perf_mode=DRM,
                        )
                        idx += 1
                nc.vector.tensor_tensor(
                    qkk_sb[:, sub, tsl],
                    pp[:],
                    bqk_sb[:, sub : sub + 1].to_broadcast((P, TCH)),
                    ADD,
                )

            def emit_kv_one(v3_sb, qkk_sb, xch, tidx, pv):
                """KV projection for one 128-token tile into psum slice pv."""
                ts = tidx % (TCH // P)
                idx = 0
                for wb, xj in TERMS:
                    for j in range(8):
                        nc.tensor.matmul(
                            pv,
                            xch[:, xj, 2 * j : 2 * j + 2, ts * P : (ts + 1) * P],
                            wkv_sb[:, wb + j, :, :],
                            start=(idx == 0),
                            stop=(idx == NDR - 1),
                            perf_mode=DRM,
                        )
                        idx += 1

            def drain_kv(v3_sb, qkk_sb, tidx, pv):
                kvn = wpool2.tile([P, 128], fp16, tag="kvn", name="kvn", bufs=4)
                nc.vector.tensor_tensor(kvn[:], pv, bkv_sb[:], ADD)
                nc.vector.tensor_copy(v3_sb[:, tidx, 0:HD], kvn[:, 64:128])
                return (tidx, kvn)

            def emit_kt(qkk_sb, tidx, kvn):
                # rebuild K^T duplicated to both partition halves via two
                # PE transposes
                pt = psumC.tile([P, TCH], f32, tag="pp", name="pt")
                pt16 = pt[:, 0:64].bitcast(fp16)
                nc.tensor.matmul(
                    pt16[0:64, :], kvn[:, 0:64], mask2_sb[:, 1, :],
                    start=True, stop=True, is_transpose=True,
                )
                nc.tensor.matmul(
                    pt16[64:128, :], kvn[:, 0:64], mask2_sb[:, 1, :],
                    start=True, stop=True, is_transpose=True,
                )
                nc.vector.tensor_copy(
                    qkk_sb[:, 2, tidx * P : (tidx + 1) * P], pt16[:]
                )

            def emit_kv_quad(v3_sb, qkk_sb, xch, tch):
                kts = []
                for ts in range(TCH // P):
                    tidx = tch * (TCH // P) + ts
                    pv = psumC.tile([P, TCH], f32, tag="pp", name="pv")
                    emit_kv_one(v3_sb, qkk_sb, xch, tidx, pv[:, 0:128])
                    kts.append(drain_kv(v3_sb, qkk_sb, tidx, pv[:, 0:128]))
                for args in kts:
                    emit_kt(qkk_sb, *args)

            def emit_p1_chunk0(qkk_sb, v3_sb):
                # k-outer across 6 psum groups so the PE can start on the
                # first DMA pieces: Q sub0/sub1 in one psA tile's two pages,
                # pv tiles in a second psA tile + the two psC slots.
                pq = psumA.tile([P, 2, QCH], f32, tag="ps", name="pq0")
                pv2 = psumA.tile([P, 2, QCH], f32, tag="ps", name="pv01")
                pc0 = psumC.tile([P, TCH], f32, tag="pp", name="pv2")
                pc1 = psumC.tile([P, TCH], f32, tag="pp", name="pv3")
                pvs = [pv2[:, 0, 0:128], pv2[:, 1, 0:128], pc0[:, 0:128], pc1[:, 0:128]]
                idx = 0
                for wb, xj in TERMS:
                    for j in range(8):
                        st = idx == 0
                        sp = idx == NDR - 1
                        for sub in range(2):
                            nc.tensor.matmul(
                                pq[:, sub, :],
                                wq_sb[:, wb + j, :, sub * P : (sub + 1) * P],
                                xch0[:, xj, 2 * j : 2 * j + 2, :],
                                start=st, stop=sp,
                                perf_mode=DRM, skip_group_check=True,
                            )
                        for ts in range(TCH // P):
                            nc.tensor.matmul(
                                pvs[ts],
                                xch0[:, xj, 2 * j : 2 * j + 2, ts * P : (ts + 1) * P],
                                wkv_sb[:, wb + j, :, :],
                                start=st, stop=sp,
                                perf_mode=DRM, skip_group_check=True,
                            )
                        idx += 1
                for sub in range(2):
                    nc.vector.tensor_tensor(
                        qkk_sb[:, sub, 0:TCH],
                        pq[:, sub, :],
                        bqk_sb[:, sub : sub + 1].to_broadcast((P, TCH)),
                        ADD,
                    )
                kts = [drain_kv(v3_sb, qkk_sb, ts, pvs[ts]) for ts in range(TCH // P)]
                for tidx, kvn in kts:
                    emit_kt(qkk_sb, tidx, kvn)

            # ---------- P3 (out-proj) fill units ----------
            DRAIN_ROT = (nc.vector, nc.scalar, nc.vector, nc.scalar)

            def make_p3_units(b, qc, attn_sb, alt_psum=False, eager=False):
                units = []
                state = {}
                for ts in range(qc * (QCH // P), (qc + 1) * (QCH // P)):
                    for ec in range(C // QCH):
                        def unit(ts=ts, ec=ec):
                            if ec == 0:
                                state["ysb"] = wpool2.tile(
                                    [P, C], fp16, tag="ysb", name="ysb",
                                    bufs=int(os.environ.get("K2_YSB", "5")),
                                )
                            if alt_psum and ec % 2 == 1:
                                py = psumA.tile(
                                    [P, 2, QCH], f32, tag="ps", name="py2"
                                )[:, 0, :]
                            else:
                                py = psumC.tile([P, QCH], f32, tag="pp", name="py")
                            for ks in range(2):
                                nc.tensor.matmul(
                                    py[:],
                                    attn_sb[:, ks, ts * P : (ts + 1) * P],
                                    wo_sb[:, ks, ec * QCH : (ec + 1) * QCH],
                                    start=(ks == 0),
                                    stop=(ks == 1),
                                    skip_group_check=True,
                                )
                            # rotate the drain across DVE/ACT/Pool to balance
                            if DRAIN_ROT[ec % 4] is nc.vector:
                                nc.vector.tensor_copy(
                                    state["ysb"][:, ec * QCH : (ec + 1) * QCH], py[:]
                                )
                            else:
                                nc.scalar.copy(
                                    state["ysb"][:, ec * QCH : (ec + 1) * QCH], py[:]
                                )
                            if eager and ts >= qc * (QCH // P) + 2:
                                # last two tiles: stream each 512-wide piece as
                                # soon as it lands (alternating queues)
                                deng = nc.gpsimd if ec % 2 == 0 else nc.sync
                                deng.dma_start(
                                    y.ap()[
                                        b,
                                        ts * P : (ts + 1) * P,
                                        ec * QCH : (ec + 1) * QCH,
                                    ],
                                    state["ysb"][:, ec * QCH : (ec + 1) * QCH],
                                )
                            elif ec == C // QCH - 1:
                                nc.sync.dma_start(
                                    y.ap()[b, ts * P : (ts + 1) * P, :], state["ysb"][:]
                                )
                        units.append(unit)
                return units

            # ---------- fill queue ----------
            fill = []
            RESERVE = int(os.environ.get("K2_RESERVE", "0"))
            STARTPOPS = int(os.environ.get("K2_STARTPOPS", "2"))
            POPMOD = int(os.environ.get("K2_POPMOD", "2"))

            def pop_fill(force=False):
                if fill and (force or len(fill) > RESERVE):
                    fill.pop(0)()

            def drain_fill():
                while fill:
                    fill.pop(0)()

            # ---------- P2: one strip = (batch, sub-pair, q-chunk) ----------
            pending = []  # deferred normalize/transpose stages

            def make_norm_stages(b, sub, qc, av, attn_sb):
                q0 = qc * QCH
                state = {}

                def s_norm():
                    rec = wpool2.tile([P, 4, 2, 1], f32, tag="rec", name="rec", bufs=2)
                    nc.vector.reciprocal(rec[:], av[:, :, :, 64:65])
                    att = wpool2.tile([P, 4, 2, 64], fp16, tag="att", name="att", bufs=2)
                    nc.vector.tensor_tensor(
                        att[:], av[:, :, :, 0:64],
                        rec[:].to_broadcast((P, 4, 2, 64)), MUL,
                    )
                    state["att"] = att

                def mk_tr(pair):
                    def s_tr():
                        att = state["att"]
                        pt = psumC.tile([P, TCH], f32, tag="pp", name="ptr")
                        for k in range(2):
                            qt = pair * 2 + k
                            dst = pt[:, 64 * k : 64 * k + 64].bitcast(fp16)
                            nc.tensor.matmul(
                                dst[:],
                                att[:, qt, :, :],
                                mask2_sb[:, 1, :],
                                start=True, stop=True, is_transpose=True,
                            )
                        state[f"pt{pair}"] = pt

                    def s_cp():
                        pt = state[f"pt{pair}"]
                        nc.vector.tensor_copy(
                            attn_sb[:, sub, q0 + pair * 2 * P : q0 + (pair * 2 + 2) * P],
                            pt[:, 0:128].bitcast(fp16),
                        )

                    return [s_tr, s_cp]

                return [s_norm] + mk_tr(0) + mk_tr(1)

            def emit_strip(b, sub, qc, qkk_sb, v3_sb, attn_sb):
                q0 = qc * QCH
                nfull = q0 // P
                ntiles = nfull + QCH // P
                av = psumB.tile([P, 4, 2, 65], f32, tag="av", name="av")
                nc.vector.memset(av[:], 0.0)
                avq = []

                def flush_av():
                    i, expS = avq.pop(0)
                    for qt in range(max(0, i - nfull), 4):
                        for h in range(2):
                            nc.tensor.matmul(
                                av[:, qt, h, :],
                                expS[:, h, qt * P : (qt + 1) * P],
                                v3_sb[:, i, :],
                                start=False,
                                stop=(i == nfull + qt),
                                skip_group_check=True,
                            )

                for step, i in enumerate(range(ntiles)):
                    nsl = slice(0, QCH) if i < nfull else slice((i - nfull) * P, QCH)
                    ksl = slice(i * P, (i + 1) * P)
                    ps_s = psumA.tile([P, 2, QCH], f32, tag="ps", name="ps_s")
                    nc.tensor.matmul(
                        ps_s[:, 0, nsl],
                        qkk_sb[0:64, 2, ksl],
                        qkk_sb[0:64, sub, q0 + nsl.start : q0 + QCH],
                        start=True,
                        stop=True,
                    )
                    nc.tensor.matmul(
                        ps_s[:, 1, nsl],
                        qkk_sb[64:128, 2, ksl],
                        qkk_sb[64:128, sub, q0 + nsl.start : q0 + QCH],
                        start=True,
                        stop=True,
                    )
                    expS = wpool.tile([P, 2, QCH], fp16, tag="expS", name="expS")
                    nc.scalar.activation(
                        expS[:, :, nsl],
                        ps_s[:, :, nsl],
                        mybir.ActivationFunctionType.Exp,
                        scale=SCL,
                    )
                    if i >= nfull:
                        j = i - nfull
                        nc.vector.tensor_tensor(
                            expS[:, :, j * P : (j + 1) * P],
                            expS[:, :, j * P : (j + 1) * P],
                            mask_sb[:, None, :].to_broadcast((P, 2, P)),
                            MUL,
                        )
                    if pending:
                        pending.pop(0)()
                        pop_fill()
                    avq.append((i, expS))
                    if len(avq) > AVLAG:
                        flush_av()
                    if step < int(os.environ.get("K2_STARTRANGE", "3")):
                        for _ in range(STARTPOPS):
                            pop_fill(force=True)
                    elif step % POPMOD == 0:
                        pop_fill()
                while avq:
                    flush_av()
                pending.extend(make_norm_stages(b, sub, qc, av, attn_sb))

            def drain_pending():
                while pending:
                    pending.pop(0)()
                    pop_fill(force=True)

            # ---------- program ----------
            qkk = {}
            v3 = {}
            attn = {}
            xchs = {}

            # P1(b=0)
            qkk[0] = projpool.tile([P, 3, T], fp16, tag="qkk", name="qkk0")
            v3[0] = projpool.tile([P, NT, 65], fp16, tag="v3", name="v30")
            nc.gpsimd.memset(v3[0][:, :, 64:65], 512.0)
            emit_p1_chunk0(qkk[0], v3[0])
            for tch in range(1, T // TCH):
                xch = xpool.tile([P, 2, KT, TCH], e4, tag="xch", name="xch")
                tsl = slice(tch * TCH, (tch + 1) * TCH)
                if tch == 1:
                    for jj in range(2):
                        nc.sync.dma_start(
                            xch[:, jj, 0:8, :], xp8.ap()[:, 0, jj, 0:8, tsl]
                        )
                        nc.sync.dma_start(
                            xch[:, jj, 8:16, :], xp8.ap()[:, 0, jj, 8:16, tsl]
                        )
                else:
                    for jj in range(2):
                        nc.sync.dma_start(xch[:, jj], xp8.ap()[:, 0, jj, :, tsl])
                if tch == 2:
                    nc.sync.dma_start(wo_sb[:], wo.ap())
                for sub in range(2):
                    emit_q_sub(qkk[0], xch, tch, sub)
                emit_kv_quad(v3[0], qkk[0], xch, tch)

            # prefetch b=1's x; build its P1 as fill units
            qkk[1] = projpool.tile([P, 3, T], fp16, tag="qkk", name="qkk1")
            v3[1] = projpool.tile([P, NT, 65], fp16, tag="v3", name="v31")
            nc.gpsimd.memset(v3[1][:, :, 64:65], 512.0)
            for tch in range(2):
                xch = xpool.tile([P, 2, KT, TCH], e4, tag="xch", name="xch")
                for jj in range(2):
                    nc.sync.dma_start(
                        xch[:, jj], xp8.ap()[:, 1, jj, :, tch * TCH : (tch + 1) * TCH]
                    )
                xchs[tch] = xch

            def p1b1_units(tch):
                xch = xchs[tch]
                us = [
                    (lambda sub=sub: emit_q_sub(qkk[1], xch, tch, sub))
                    for sub in range(2)
                ]
                us.append(lambda: emit_kv_quad(v3[1], qkk[1], xch, tch))
                return us

            # P2/P3 (b=0): fill = P1(b=1) chunk 0 + b=0 out-proj tiles
            attn[0] = apool.tile([P, 2, T], fp16, tag="attn", name="attn0")
            heldover0 = []
            c0units = p1b1_units(0)
            for qc in range(T // QCH):
                if qc < len(c0units):
                    fill.append(c0units[qc])
                if qc >= 2:
                    xch = xpool.tile([P, 2, KT, TCH], e4, tag="xch", name="xch")
                    for jj in range(2):
                        nc.sync.dma_start(
                            xch[:, jj], xp8.ap()[:, 1, jj, :, qc * TCH : (qc + 1) * TCH]
                        )
                    xchs[qc] = xch
                fill.extend(heldover0)
                heldover0 = []
                emit_strip(0, 0, qc, qkk[0], v3[0], attn[0])
                emit_strip(0, 1, qc, qkk[0], v3[0], attn[0])
                if qc > 0:
                    us = make_p3_units(0, qc - 1, attn[0])
                    h0 = 16 - int(os.environ.get("K2_HOLD0", "0"))
                    fill.extend(us[:h0])
                    heldover0 = us[h0:]
            drain_pending()
            fill.extend(heldover0)
            tailb0 = make_p3_units(0, 3, attn[0])

            # P2/P3 (b=1): strip qc only needs projection chunks <= qc, so
            # P1(b=1) chunks c1..c3 serve as fill inside qc0..qc2
            attn[1] = apool.tile([P, 2, T], fp16, tag="attn", name="attn1")
            heldover = []
            for qc in range(T // QCH):
                if qc == 0:
                    fill[:0] = p1b1_units(1)
                elif qc < 3:
                    fill[:0] = p1b1_units(qc + 1)
                else:
                    fill.extend(tailb0)
                fill.extend(heldover)
                heldover = []
                emit_strip(1, 0, qc, qkk[1], v3[1], attn[1])
                emit_strip(1, 1, qc, qkk[1], v3[1], attn[1])
                if qc > 0:
                    us = make_p3_units(1, qc - 1, attn[1])
                    h1 = 16 - int(os.environ.get("K2_HOLD1", "8"))
                    fill.extend(us[:h1])
                    heldover = us[h1:]
            fill.extend(heldover)
            drain_pending()
            drain_fill()
            for u in make_p3_units(1, 3, attn[1], alt_psum=True, eager=True):
                u()

    nc.compile()
    return nc


def _hilo8(a):
    """Split float array into e4m3 hi + lo (of the residual)."""
    a = np.asarray(a, dtype=np.float32)
    hi = a.astype(ml_dtypes.float8_e4m3)
    lo = (a - hi.astype(np.float32)).astype(ml_dtypes.float8_e4m3)
    return hi, lo


def _prep_inputs(x, Wq, bq, Wk, bk, Wv, bv, Wo, bo):
    x = np.ascontiguousarray(np.asarray(x, dtype=np.float32))
    Wq = np.asarray(Wq, dtype=np.float32)
    Wk = np.asarray(Wk, dtype=np.float32)
    Wv = np.asarray(Wv, dtype=np.float32)
    Wo = np.asarray(Wo, dtype=np.float32)
    bq = np.asarray(bq, dtype=np.float32)
    bk = np.asarray(bk, dtype=np.float32)
    bv = np.asarray(bv, dtype=np.float32)

    # xp8[p, b, j, k, t] = hi/lo e4m3 of 8*x[b, t, k*128+p]
    xkpt = x.transpose(0, 2, 1).reshape(B, KT, P, T)
    xhi, xlo = _hilo8(8.0 * xkpt)
    xp8 = np.ascontiguousarray(
        np.stack([xhi, xlo], axis=1).transpose(3, 0, 1, 2, 4)
    )  # [P, B, 2, KT, T]

    # page 0: mask[kj, qi] = 1 iff kj <= qi; page 1: identity (PE transposes)
    mask = np.ascontiguousarray(
        np.stack(
            [np.triu(np.ones((P, P), np.float16)), np.eye(P, dtype=np.float16)],
            axis=1,
        )
    )

    def pack_w(Wsl, width):
        """W [2048, width] -> [P, 16, 2, width] hi/lo k-pair layout."""
        hi, lo = _hilo8(64.0 * Wsl)
        out = np.empty((P, 16, 2, width), dtype=ml_dtypes.float8_e4m3)
        hik = hi.reshape(KT, P, width)
        lok = lo.reshape(KT, P, width)
        for j in range(8):
            out[:, j, 0] = hik[2 * j]
            out[:, j, 1] = hik[2 * j + 1]
            out[:, 8 + j, 0] = lok[2 * j]
            out[:, 8 + j, 1] = lok[2 * j + 1]
        return np.ascontiguousarray(out)

    in_maps = []
    for c in range(NCORES):
        qs = slice(c * G * HD, (c + 1) * G * HD)
        ks = slice(c * HD, (c + 1) * HD)
        wq_p = pack_w(Wq[:, qs], 256)
        wkv_p = pack_w(np.concatenate([Wk[:, ks], Wv[:, ks]], axis=1), 128)
        wo_p = Wo[qs, :].reshape(2, P, C).transpose(1, 0, 2)
        bq_c = 512.0 * bq[qs]
        bqk_c = np.stack([bq_c[0:128], bq_c[128:256]], axis=1)
        bkv_c = 512.0 * np.concatenate([bk[ks], bv[ks]])
        in_maps.append(
            {
                "xp8": xp8,
                "wq8": wq_p,
                "wkv8": wkv_p,
                "wo": np.ascontiguousarray(wo_p).astype(np.float16),
                "bqk": np.ascontiguousarray(bqk_c),
                "bkv": np.ascontiguousarray(bkv_c[None, :]),
                "mask": mask,
            }
        )
    return in_maps


def kernel(x, Wq, bq, Wk, bk, Wv, bv, Wo, bo, _trace=False):
    if not _trace:
        os.environ["BASS_NEVER_TRACE"] = "1"
    if "nc" not in _CACHE:
        _CACHE["nc"] = _build()
    nc = _CACHE["nc"]
    in_maps = _prep_inputs(x, Wq, bq, Wk, bk, Wv, bv, Wo, bo)
    res = bass_utils.run_bass_kernel_spmd(
        nc, in_maps, core_ids=list(range(NCORES)), trace=_trace
    )
    bo = np.asarray(bo, dtype=np.float32)
    y = np.zeros((B, T, C), dtype=np.float32)
    for c in range(NCORES):
        y += res.results[c]["y"].astype(np.float32)
    y += bo
    if _trace:
        return y, res
    return y
